# revision 35
# baseline (speedup 1.0000x reference)
"""BloomBlock (B=1, S=2048, H=2048, NH=16) on 8 Trainium2 NeuronCores.

Megatron tensor-parallel: each core owns 2 attention heads and 1024 rows of
the 8192-wide MLP. LN1 (+transpose) is replicated; attention/MLP partial sums
are reduce-scattered over the sequence in bf16; LN2 runs on the local
sequence shard; normalized activations are all-gathered (transposed) for the
MLP. The final output stays sequence-sharded; the host reassembles.

All collectives are chunked by sequence groups of 512 rows (4 chunks) so
they overlap compute: attention+dense for group g is followed immediately by
ReduceScatter chunk g (hidden under group g+1's attention); LN2/AllGather
for the first two strips run mid-attention; fc1/fc2 run per group with RS2
chunk g issued behind fc2 group g. Row-chunked ReduceScatter gives core c
ownership of 4 strips of 64 rows (rows g*512 + c*64 + [0,64)); the host
maps res1/out accordingly.

All matmul operands are bf16 (fp32 PSUM accumulation); LayerNorm, softmax
statistics, alibi, residuals and final outputs are fp32.

Activations are kept transposed [feature, seq] so every matmul contracts
along the partition axis:
  hid --LN1--> xhat --PE transpose--> xT
  qT,kT = Wqk @ xT            v = xT.T @ WvT   (natural [s,d], lhsT for ctx)
  scores = qT.T @ kT  (+alibi fused w/ row-max, causal mask, ACT exp+rowsum)
  wT = probs.T @ diag(1/rowsum)   (PE transpose-normalize)
  ctxT = v.T @ wT  (+v bias)
  dense partial = ctxT.T @ WdT --> bf16 ReduceScatter --> + residual1
  LN2 --> yT strips --> bf16 AllGather --> hdnT = gelu(Wf1 @ yT + b)
  mlp partial = hdnT.T @ Wf2T --> bf16 ReduceScatter --> + residual2 --> out
"""
import sys

for _p in ("/opt/trn_rl_repo",):
    if _p not in sys.path:
        sys.path.insert(0, _p)

import numpy as np
import ml_dtypes

import concourse.bass as bass
from concourse import bacc
import concourse.mybir as mybir
import concourse.tile as tile
from concourse.bass_utils import run_bass_kernel_spmd
from concourse.masks import make_identity, make_causal_mask

AF = mybir.ActivationFunctionType
ALU = mybir.AluOpType
AX = mybir.AxisListType

B, S, H, NH, HD = 1, 2048, 2048, 16, 128
NCORE = 8
NHC = NH // NCORE          # heads per core = 2
SSH = S // NCORE           # sequence shard = 256
F1 = 4 * H                 # 8192
F1C = F1 // NCORE          # 1024
NSB = S // 128             # 16 s-blocks
NHCH = H // 128            # 16 h-chunks
NG = 4                     # sequence groups (collective chunks), uneven
GRP = [(0, 1024), (1024, 512), (1536, 256), (1792, 256)]  # (start, width)
STW = [w // NCORE for _, w in GRP]                        # strip widths
SB2G = []                   # s-block -> (group, local row offset)
for _sb in range(NSB):
    for _g, (_st, _w) in enumerate(GRP):
        if _st <= _sb * 128 < _st + _w:
            SB2G.append((_g, _sb * 128 - _st))
            break
EPS = 1e-5
MASK_NEG = -1e30

FP = mybir.dt.float32
BF = mybir.dt.bfloat16

SIM_MODE = False           # CoreSim lacks Gelu; use Identity there
PANEL = 256                # stage-A transpose/QKV panel width
NPANEL = S // PANEL


def _ln_stats(nc, stats, work, src, eps_t, scratch_tag="scsb",
              scratch=None):
    """Mean/rstd over the free axis of src [128, H]. Returns (rstd, -mu*rstd)."""
    ssum = stats.tile([128, 1], FP, tag="ssum")
    nc.vector.reduce_sum(ssum, src, axis=AX.X)
    mu = stats.tile([128, 1], FP, tag="mu")
    nc.vector.tensor_scalar_mul(mu, ssum, 1.0 / H)
    sq = scratch if scratch is not None else work.tile(
        [128, H], FP, tag=scratch_tag, bufs=2, name="sq")
    ssq = stats.tile([128, 1], FP, tag="ssq")
    nc.scalar.activation(sq, src, AF.Square, accum_out=ssq)
    msq = stats.tile([128, 1], FP, tag="msq")
    nc.vector.tensor_scalar_mul(msq, ssq, 1.0 / H)
    mu2 = stats.tile([128, 1], FP, tag="mu2")
    nc.vector.tensor_mul(mu2, mu, mu)
    var = stats.tile([128, 1], FP, tag="var")
    nc.vector.tensor_sub(var, msq, mu2)
    std = stats.tile([128, 1], FP, tag="std")
    nc.scalar.activation(std, var, AF.Sqrt, bias=eps_t)
    rstd = stats.tile([128, 1], FP, tag="rstd")
    nc.vector.reciprocal(rstd, std)
    nmurs = stats.tile([128, 1], FP, tag="nmurs")
    nc.vector.tensor_mul(nmurs, mu, rstd)
    nc.vector.tensor_scalar_mul(nmurs, nmurs, -1.0)
    return rstd, nmurs


def build_program():
    nc = bacc.Bacc("TRN2", target_bir_lowering=False, debug=False,
                   enable_asserts=True, num_devices=NCORE)

    # ---------------- I/O ----------------
    hid = nc.declare_dram_parameter("hid", [S, H], BF, isOutput=False)
    wqk = nc.declare_dram_parameter("wqk", [H, 4 * HD], BF, isOutput=False)
    bqk = nc.declare_dram_parameter("bqk", [4, HD], FP, isOutput=False)
    wv = nc.declare_dram_parameter("wv", [H, NHC * HD], BF, isOutput=False)
    bv = nc.declare_dram_parameter("bv", [NHC, HD], FP, isOutput=False)
    alib = nc.declare_dram_parameter("alib", [NHC, S], FP, isOutput=False)
    wd = nc.declare_dram_parameter("wd", [NHC * HD, H], BF, isOutput=False)
    res1 = nc.declare_dram_parameter("res1", [SSH, H], FP, isOutput=False)
    wf1 = nc.declare_dram_parameter("wf1", [H, F1C], BF, isOutput=False)
    bf1 = nc.declare_dram_parameter("bf1", [F1C // 128, 128], FP, isOutput=False)
    wf2 = nc.declare_dram_parameter("wf2", [F1C, H], BF, isOutput=False)
    fc2b = nc.declare_dram_parameter("fc2b", [128, H], BF, isOutput=False)
    out = nc.declare_dram_parameter("out", [SSH, H], FP, isOutput=True)

    rg = [list(range(NCORE))]

    with tile.TileContext(nc) as tc:
        with (
            tc.tile_pool(name="dram", bufs=1, space="DRAM") as dram,
            tc.tile_pool(name="consts", bufs=1) as consts,
            tc.tile_pool(name="stats", bufs=4) as stats,
            tc.tile_pool(name="pmm", bufs=1, space="PSUM") as pmm,
            tc.tile_pool(name="psc", bufs=2, space="PSUM") as psc,
            tc.tile_pool(name="psw", bufs=1, space="PSUM") as psw,
            tc.tile_pool(name="stageE", bufs=1) as stageE,
            tc.tile_pool(name="workE", bufs=2) as workE,
        ):
            # ---------------- constants ----------------
            ident = consts.tile([128, 128], BF, tag="ident")
            make_identity(nc, ident)
            cmask = consts.tile([128, 128], FP, tag="cmask")
            make_causal_mask(nc, cmask, mask_val=MASK_NEG)
            bqk_t = consts.tile([128, 4], FP, tag="bqk")
            nc.sync.dma_start(out=bqk_t, in_=bqk[:, :].rearrange("b p -> p b"))
            bv_t = consts.tile([128, NHC], FP, tag="bv")
            nc.sync.dma_start(out=bv_t, in_=bv[:, :].rearrange("b p -> p b"))
            bf1_t = consts.tile([128, F1C // 128], FP, tag="bf1")
            nc.sync.dma_start(out=bf1_t, in_=bf1[:, :].rearrange("b p -> p b"))
            eps_t = consts.tile([128, 1], FP, tag="eps")
            nc.vector.memset(eps_t, EPS)
            fc2b_t = consts.tile([128, H], BF, tag="fc2b")
            nc.sync.dma_start(out=fc2b_t, in_=fc2b[:, :])

            # collective bounce buffers, chunked by uneven sequence groups;
            # core c receives a STW[g]-row strip per chunk
            rs1_in = [dram.tile([GRP[g][1], H], BF, tag=f"rs1i{g}",
                                name=f"rs1i{g}") for g in range(NG)]
            rs1_out = [dram.tile([STW[g], H], BF, tag=f"rs1o{g}",
                                 name=f"rs1o{g}") for g in range(NG)]
            ag_in = [dram.tile([H, STW[g]], BF, tag=f"agi{g}", name=f"agi{g}")
                     for g in range(NG)]
            ag_out = [dram.tile([NCORE, H, STW[g]], BF, tag=f"ago{g}",
                                name=f"ago{g}", addr_space="Shared")
                      for g in range(NG)]
            rs2_in = [dram.tile([GRP[g][1], H], BF, tag=f"rs2i{g}",
                                name=f"rs2i{g}") for g in range(NG)]
            rs2_out = [dram.tile([STW[g], H], BF, tag=f"rs2o{g}",
                                 name=f"rs2o{g}") for g in range(NG)]

            # persistent stage-E state (coexists with attention pools)
            attn = stageE.tile([128, 2, H], FP, tag="attn")

            # tile 0 = group-0 strip (128 rows); tile 1 = strips 1,2,3
            TILE_GROUPS = [[0], [1, 2, 3]]

            def emit_ln2_tile(t):
                """LN2 + transpose for the tile's strips; then AG chunks."""
                r1b = workE.tile([128, H], FP, tag="res1blk", name="r1b",
                                 bufs=1)
                nc.sync.dma_start(out=r1b,
                                  in_=res1[t * 128:(t + 1) * 128, :])
                rsb = workE.tile([128, H], BF, tag="rs1blk", name="rsb")
                po = 0
                for g in TILE_GROUPS[t]:
                    nc.sync.dma_start(out=rsb[po:po + STW[g], :],
                                      in_=rs1_out[g])
                    po += STW[g]
                nc.vector.tensor_add(attn[:, t, :], rsb, r1b)
                rstd, nmurs = _ln_stats(nc, stats, workE, attn[:, t, :],
                                        eps_t, scratch=r1b)
                yh = workE.tile([128, H], BF, tag="yhat", bufs=1)
                nc.scalar.activation(yh, attn[:, t, :], AF.Identity,
                                     bias=nmurs, scale=rstd)
                yT = workE.tile([128, NHCH, 128], BF, tag="yT", bufs=1)
                for hc in range(NHCH):
                    pt = pmm.tile([128, 128], BF, tag="mmT", bufs=1)
                    nc.tensor.transpose(pt, yh[:, hc * 128:(hc + 1) * 128],
                                        ident)
                    nc.scalar.copy(yT[:, hc, :], pt)
                po = 0
                for g in TILE_GROUPS[t]:
                    nc.sync.dma_start(
                        out=ag_in[g][:, :].rearrange("(c p) s -> p c s", p=128),
                        in_=yT[:, :, po:po + STW[g]])
                    po += STW[g]
                    nc.gpsimd.collective_compute(
                        "AllGather", ALU.bypass, replica_groups=rg,
                        ins=[ag_in[g].opt()], outs=[ag_out[g].opt()])

            # ======== attention-lifetime scope (stages A-D) ========
            with (
                tc.tile_pool(name="attnp", bufs=1) as attnp,
                tc.tile_pool(name="workA", bufs=2) as workA,
            ):
                # first panel's hidden blocks first so LN1 starts immediately
                hb_pre = []
                for sb in range(PANEL // 128):
                    hb = workA.tile([128, H], BF, tag="hidblk", name="hb")
                    nc.sync.dma_start(out=hb, in_=hid[sb * 128:(sb + 1) * 128, :])
                    hb_pre.append(hb)

                alibi_bc = attnp.tile([128, NHC, S], FP, tag="alibi")
                for h in range(NHC):
                    astage = workA.tile([128, S], FP, tag="scsb", name="astage")
                    nc.sync.dma_start(out=astage[0:1, :], in_=alib[h:h + 1, :])
                    nc.gpsimd.partition_broadcast(
                        alibi_bc[:, h, :], astage[0:1, :])
                # upper-bound row max: m[q] = alibi[q] + 8 (alibi rises in k,
                # so the true max of s+alibi is within ~8 of alibi[q])
                negm = attnp.tile([128, NHC * NSB], FP, tag="negm")
                acol = workA.tile([128, NHC * NSB], FP, tag="acol", bufs=1)
                nc.sync.dma_start(
                    out=acol,
                    in_=alib[:, :].rearrange("h (b p) -> p (h b)", p=128))
                nc.vector.tensor_scalar(negm, acol, -1.0, -8.0,
                                        ALU.mult, ALU.add)
                wd_t = attnp.tile([128, NHC, H], BF, tag="wd")
                for h in range(NHC):
                    nc.sync.dma_start(out=wd_t[:, h, :],
                                      in_=wd[h * 128:(h + 1) * 128, :])
                wqk_t = attnp.tile([128, NHCH, 4 * HD], BF, tag="wqk")
                wv_t = attnp.tile([128, NHCH, NHC * HD], BF, tag="wv")
                for hc in range(NHCH):
                    nc.sync.dma_start(out=wqk_t[:, hc, :],
                                      in_=wqk[hc * 128:(hc + 1) * 128, :])
                    nc.sync.dma_start(out=wv_t[:, hc, :],
                                      in_=wv[hc * 128:(hc + 1) * 128, :])

                qkT = attnp.tile([128, 2, NHC, S], BF, tag="qkT")
                v_t = attnp.tile([128, NSB, NHC * HD], BF, tag="v")
                ctxT = attnp.tile([128, NHC, S], BF, tag="ctxT")

                # ---- Stage A+B: LN1, transpose, QKV per panel ----
                for p in range(NPANEL):
                    xhat_blocks = []
                    for sb in range(PANEL // 128):
                        s0 = p * PANEL + sb * 128
                        if p == 0:
                            hb = hb_pre[sb]
                        else:
                            hb = workA.tile([128, H], BF, tag="hidblk",
                                            name="hb")
                            nc.sync.dma_start(out=hb, in_=hid[s0:s0 + 128, :])
                        rstd, nmurs = _ln_stats(nc, stats, workA, hb, eps_t)
                        xh = workA.tile([128, H], BF, tag="xhat", bufs=3)
                        nc.scalar.activation(xh, hb, AF.Identity,
                                             bias=nmurs, scale=rstd)
                        xhat_blocks.append(xh)

                    xT = workA.tile([128, NHCH, PANEL], BF, tag="xT", bufs=1)
                    for hc in range(NHCH):
                        pt = pmm.tile([128, PANEL], BF, tag="mmT", bufs=1)
                        for sb in range(PANEL // 128):
                            nc.tensor.transpose(
                                pt[:, sb * 128:(sb + 1) * 128],
                                xhat_blocks[sb][:, hc * 128:(hc + 1) * 128],
                                ident)
                        nc.vector.tensor_copy(xT[:, hc, :], pt)

                    for fb in range(4):  # q_h0, k_h0, q_h1, k_h1
                        pq = pmm.tile([128, PANEL], FP, tag="mm", bufs=3)
                        for hc in range(NHCH):
                            nc.tensor.matmul(
                                pq, wqk_t[:, hc, fb * 128:(fb + 1) * 128],
                                xT[:, hc, :],
                                start=(hc == 0), stop=(hc == NHCH - 1))
                        head, isk = fb // 2, fb % 2
                        nc.scalar.activation(
                            qkT[:, isk, head, p * PANEL:(p + 1) * PANEL], pq,
                            AF.Identity, bias=bqk_t[:, fb:fb + 1], scale=1.0)

                    for sb in range(PANEL // 128):
                        blk = p * (PANEL // 128) + sb
                        pv = pmm.tile([128, NHC * HD], FP, tag="mm", bufs=3)
                        for hc in range(NHCH):
                            nc.tensor.matmul(
                                pv, xT[:, hc, sb * 128:(sb + 1) * 128],
                                wv_t[:, hc, :],
                                start=(hc == 0), stop=(hc == NHCH - 1))
                        nc.vector.tensor_copy(v_t[:, blk, :], pv)

                # ---- Stage C+D: attention, dense, RS1 per group ----
                for g in range(NG):
                    for head in range(NHC):
                        probs_g = []
                        dn_g = []
                        for ib in range(4 * g, 4 * g + 4):
                            jw = (ib + 1) * 128
                            nchunk = (jw + 511) // 512
                            probs = workA.tile([128, S], BF, tag="probs", bufs=5)
                            ssc = workA.tile([128, S], FP, tag="scsb", bufs=2,
                                             name="ssc")
                            nmc = negm[:, head * NSB + ib:head * NSB + ib + 1]
                            dparts = []
                            for jc in range(nchunk):
                                j0 = jc * 512
                                w_ = min(512, jw - j0)
                                ps = psc.tile([128, 512], FP, tag="sc")
                                nc.tensor.matmul(
                                    ps[:, :w_],
                                    qkT[:, 0, head, ib * 128:(ib + 1) * 128],
                                    qkT[:, 1, head, j0:j0 + w_],
                                    start=True, stop=True)
                                if jc == nchunk - 1:
                                    off = ib * 128 - j0
                                    nc.vector.tensor_add(
                                        ps[:, off:off + 128],
                                        ps[:, off:off + 128], cmask)
                                nc.vector.tensor_add(
                                    ssc[:, j0:j0 + w_], ps[:, :w_],
                                    alibi_bc[:, head, j0:j0 + w_])
                                dpart = stats.tile([128, 1], FP,
                                                   tag=f"dp{jc}", name="dp")
                                nc.scalar.activation(
                                    probs[:, j0:j0 + w_], ssc[:, j0:j0 + w_],
                                    AF.Exp,
                                    bias=nmc, scale=1.0, accum_out=dpart)
                                dparts.append(dpart)
                            denom = dparts[0]
                            for dp in dparts[1:]:
                                dnew = stats.tile([128, 1], FP, tag="denom")
                                nc.vector.tensor_add(dnew, denom, dp)
                                denom = dnew
                            invd = stats.tile([128, 1], FP, tag="invd")
                            nc.vector.reciprocal(invd, denom)
                            dn = workA.tile([128, 128], BF, tag="dn", bufs=5)
                            nc.vector.tensor_scalar_mul(dn, ident, invd)
                            probs_g.append(probs)
                            dn_g.append(dn)

                        pctx = psw.tile([128, 512], FP, tag="ctxacc", bufs=1)
                        njc = 4 * g + 4
                        for jc in range(njc):
                            ib0 = max(jc, 4 * g)
                            nblk = 4 * g + 4 - ib0
                            pw = psw.tile([128, 512], FP, tag="wt")
                            for k, ib in enumerate(range(ib0, 4 * g + 4)):
                                nc.tensor.matmul(
                                    pw[:, k * 128:(k + 1) * 128],
                                    probs_g[ib - 4 * g][:, jc * 128:(jc + 1) * 128],
                                    dn_g[ib - 4 * g],
                                    start=True, stop=True)
                            wts = workA.tile([128, 512], BF, tag="wts", bufs=2)
                            nc.vector.tensor_copy(wts[:, :nblk * 128],
                                                  pw[:, :nblk * 128])
                            off = (ib0 - 4 * g) * 128
                            nc.tensor.matmul(
                                pctx[:, off:off + nblk * 128],
                                v_t[:, jc, head * HD:(head + 1) * HD],
                                wts[:, :nblk * 128],
                                start=(jc == 0), stop=(jc == njc - 1),
                                skip_group_check=True)
                        nc.scalar.activation(
                            ctxT[:, head, g * 512:(g + 1) * 512], pctx,
                            AF.Identity, bias=bv_t[:, head:head + 1], scale=1.0)

                    # dense partials for this subgroup's 512 rows; RS1
                    # chunks fire as uneven groups complete
                    for sbl in range(4):
                        sb = 4 * g + sbl
                        gi, loc = SB2G[sb]
                        for fc in range(H // 512):
                            pd = pmm.tile([128, 512], FP, tag="mm", bufs=3)
                            for h in range(NHC):
                                nc.tensor.matmul(
                                    pd, ctxT[:, h, sb * 128:(sb + 1) * 128],
                                    wd_t[:, h, fc * 512:(fc + 1) * 512],
                                    start=(h == 0), stop=(h == NHC - 1))
                            dsb = workA.tile([128, 512], BF, tag="densebf",
                                             bufs=4)
                            nc.scalar.copy(dsb, pd)
                            nc.sync.dma_start(
                                out=rs1_in[gi][loc:loc + 128,
                                               fc * 512:(fc + 1) * 512],
                                in_=dsb)
                        if (sb + 1) * 128 == GRP[gi][0] + GRP[gi][1] \
                                and gi < 2:
                            nc.gpsimd.collective_compute(
                                "ReduceScatter", ALU.add, replica_groups=rg,
                                ins=[rs1_in[gi].opt()],
                                outs=[rs1_out[gi].opt()])

                    if g == 2:
                        # strip 0 landed after subgroup 1; LN2 tile 0 + AG
                        # chunk 0 run while subgroup 3 attention computes
                        emit_ln2_tile(0)

            # ======== Stage E tail + F: MLP per group + RS2 ========
            with (
                tc.tile_pool(name="postp", bufs=1) as postp,
                tc.tile_pool(name="workF", bufs=2) as workF,
            ):
                nf1 = F1C // 128  # 8
                wf1_t = postp.tile([128, NHCH, F1C], BF, tag="wf1")
                for hc in range(NHCH):
                    nc.sync.dma_start(out=wf1_t[:, hc, :],
                                      in_=wf1[hc * 128:(hc + 1) * 128, :])
                def emit_fc1_chunk(g, sc):
                    W, SW = GRP[g][1], STW[g]
                    wcs = min(512, W - sc * 512)
                    c0 = sc * 512 // SW
                    ncs = wcs // SW
                    yTs = workF.tile([128, NHCH, 512], BF, tag="yTs",
                                     name="yTs", bufs=2)
                    for hc in range(NHCH):
                        nc.sync.dma_start(
                            out=yTs[:, hc, 0:wcs].rearrange(
                                "p (k s) -> p k s", k=ncs),
                            in_=ag_out[g][c0:c0 + ncs,
                                          hc * 128:(hc + 1) * 128,
                                          :].rearrange("k p s -> p k s"))
                    hdn = workF.tile([128, nf1, 512], BF, tag="hdnT",
                                     name="hdn", bufs=2)
                    for f1c in range(nf1):
                        pf = pmm.tile([128, 512], FP, tag="mm", bufs=3)
                        for hc in range(NHCH):
                            nc.tensor.matmul(
                                pf[:, :wcs],
                                wf1_t[:, hc, f1c * 128:(f1c + 1) * 128],
                                yTs[:, hc, 0:wcs],
                                start=(hc == 0), stop=(hc == NHCH - 1))
                        nc.scalar.activation(
                            hdn[:, f1c, 0:wcs], pf[:, :wcs],
                            AF.Identity if SIM_MODE else AF.Gelu_apprx_tanh,
                            bias=bf1_t[:, f1c:f1c + 1], scale=1.0)
                    return hdn, wcs

                def emit_fc2_chunk(g, sc, hdn, wcs):
                    for kk in range(wcs // 128):
                        r0 = sc * 512 + kk * 128
                        for fc in range(H // 512):
                            pm = pmm.tile([128, 512], FP, tag="mm", bufs=3)
                            for f1c in range(nf1):
                                nc.tensor.matmul(
                                    pm,
                                    hdn[:, f1c, kk * 128:(kk + 1) * 128],
                                    wf2_t[:, f1c, fc * 512:(fc + 1) * 512],
                                    start=(f1c == 0), stop=(f1c == nf1 - 1))
                            msb = workF.tile([128, 512], BF, tag="mlpbf",
                                             bufs=3)
                            nc.vector.tensor_copy(msb, pm)
                            nc.sync.dma_start(
                                out=rs2_in[g][r0:r0 + 128,
                                              fc * 512:(fc + 1) * 512],
                                in_=msb)

                def emit_rs2(g):
                    nc.gpsimd.collective_compute(
                        "ReduceScatter", ALU.add, replica_groups=rg,
                        ins=[rs2_in[g].opt()], outs=[rs2_out[g].opt()])

                def emit_final(t):
                    rsb2 = workE.tile([128, H], BF, tag="rs1blk",
                                      name="rsb2")
                    po = 0
                    for g in TILE_GROUPS[t]:
                        nc.sync.dma_start(out=rsb2[po:po + STW[g], :],
                                          in_=rs2_out[g])
                        po += STW[g]
                    ob = workE.tile([128, H], FP, tag="outblk", bufs=1)
                    nc.vector.tensor_add(ob, rsb2, attn[:, t, :])
                    nc.vector.tensor_add(ob, ob, fc2b_t)
                    nc.sync.dma_start(out=out[t * 128:(t + 1) * 128, :],
                                      in_=ob)

                h, w = emit_fc1_chunk(0, 0)
                # RS1 chunks 2,3 fire only after fc1(0,0)'s input DMAs are
                # enqueued: collective traffic shares the DMA queues
                for gi in (2, 3):
                    nc.gpsimd.collective_compute(
                        "ReduceScatter", ALU.add, replica_groups=rg,
                        ins=[rs1_in[gi].opt()], outs=[rs1_out[gi].opt()])
                wf2_t = postp.tile([128, nf1, H], BF, tag="wf2")
                for f1c in range(nf1):
                    nc.sync.dma_start(out=wf2_t[:, f1c, :],
                                      in_=wf2[f1c * 128:(f1c + 1) * 128, :])
                emit_ln2_tile(1)
                emit_fc2_chunk(0, 0, h, w)
                h, w = emit_fc1_chunk(0, 1)
                emit_fc2_chunk(0, 1, h, w)
                emit_rs2(0)
                emit_final(0)
                for g in range(1, NG):
                    h, w = emit_fc1_chunk(g, 0)
                    emit_fc2_chunk(g, 0, h, w)
                    emit_rs2(g)
                emit_final(1)
    nc.compile()
    return nc


def _host_prep(inputs):
    """Slice/fold weights per core. Returns list of per-core input maps."""
    bf16 = ml_dtypes.bfloat16
    hs = np.asarray(inputs["hidden_states"], np.float32).reshape(S, H)
    g1 = np.asarray(inputs["ln1_g"], np.float32)
    b1 = np.asarray(inputs["ln1_b"], np.float32)
    qkv_w = np.asarray(inputs["qkv_w"], np.float32)
    qkv_b = np.asarray(inputs["qkv_b"], np.float32)
    dense_w = np.asarray(inputs["dense_w"], np.float32)
    dense_b = np.asarray(inputs["dense_b"], np.float32)
    g2 = np.asarray(inputs["ln2_g"], np.float32)
    b2 = np.asarray(inputs["ln2_b"], np.float32)
    fc1_w = np.asarray(inputs["fc1_w"], np.float32)
    fc1_b = np.asarray(inputs["fc1_b"], np.float32)
    fc2_w = np.asarray(inputs["fc2_w"], np.float32)
    fc2_b = np.asarray(inputs["fc2_b"], np.float32)
    alibi = np.asarray(inputs["alibi"], np.float32).reshape(NH, S)

    inv = 1.0 / np.sqrt(np.float32(HD))
    in_maps = []
    for c in range(NCORE):
        heads = [NHC * c + i for i in range(NHC)]
        wqk_cols, bqk_rows, wv_cols, bv_rows = [], [], [], []
        for h in heads:
            qr = qkv_w[h * 3 * HD:h * 3 * HD + HD, :]
            kr = qkv_w[h * 3 * HD + HD:h * 3 * HD + 2 * HD, :]
            vr = qkv_w[h * 3 * HD + 2 * HD:h * 3 * HD + 3 * HD, :]
            qb = qkv_b[h * 3 * HD:h * 3 * HD + HD] + qr @ b1
            kb = qkv_b[h * 3 * HD + HD:h * 3 * HD + 2 * HD] + kr @ b1
            vb = qkv_b[h * 3 * HD + 2 * HD:h * 3 * HD + 3 * HD] + vr @ b1
            wqk_cols.append((qr * g1[None, :]).T * inv)
            wqk_cols.append((kr * g1[None, :]).T)
            bqk_rows.append(qb * inv)
            bqk_rows.append(kb)
            wv_cols.append((vr * g1[None, :]).T)
            bv_rows.append(vb)
        # core c's strips: rows GRP[g][0] + c*STW[g] + [0, STW[g])
        strips = np.concatenate(
            [hs[GRP[g][0] + c * STW[g]: GRP[g][0] + (c + 1) * STW[g], :]
             for g in range(NG)], axis=0)
        in_maps.append({
            "hid": hs.astype(bf16),
            "wqk": np.ascontiguousarray(
                np.concatenate(wqk_cols, axis=1)).astype(bf16),
            "bqk": np.ascontiguousarray(np.stack(bqk_rows, axis=0)),
            "wv": np.ascontiguousarray(
                np.concatenate(wv_cols, axis=1)).astype(bf16),
            "bv": np.ascontiguousarray(np.stack(bv_rows, axis=0)),
            "alib": np.ascontiguousarray(alibi[heads[0]:heads[-1] + 1, :]),
            "wd": np.ascontiguousarray(
                dense_w[:, heads[0] * HD:(heads[-1] + 1) * HD].T).astype(bf16),
            "res1": np.ascontiguousarray(strips + dense_b[None, :]),
            "wf1": np.ascontiguousarray(
                (fc1_w[c * F1C:(c + 1) * F1C, :] * g2[None, :]).T).astype(bf16),
            "bf1": np.ascontiguousarray(
                (fc1_b[c * F1C:(c + 1) * F1C]
                 + fc1_w[c * F1C:(c + 1) * F1C, :] @ b2
                 ).reshape(F1C // 128, 128)),
            "wf2": np.ascontiguousarray(
                fc2_w[:, c * F1C:(c + 1) * F1C].T).astype(bf16),
            "fc2b": np.ascontiguousarray(
                np.broadcast_to(fc2_b.astype(bf16), (128, H))),
        })
    return in_maps


def _assemble(shards):
    """Reassemble strip-owned shards (uneven groups)."""
    full = np.empty((S, H), np.float32)
    for c, sh in enumerate(shards):
        sh = np.asarray(sh, np.float32)
        po = 0
        for g in range(NG):
            full[GRP[g][0] + c * STW[g]: GRP[g][0] + (c + 1) * STW[g], :] = \
                sh[po:po + STW[g]]
            po += STW[g]
    return full.reshape(B, S, H)


_CACHED_NC = None


_WARMED = False


def kernel(**inputs) -> np.ndarray:
    global _CACHED_NC, _WARMED
    in_maps = _host_prep(inputs)
    if _CACHED_NC is None:
        _CACHED_NC = build_program()
    if not _WARMED:
        # first execution after NEFF load has been observed to race on
        # collective warmup; run once and discard
        run_bass_kernel_spmd(_CACHED_NC, in_maps, list(range(NCORE)))
        _WARMED = True
    res = run_bass_kernel_spmd(_CACHED_NC, in_maps, list(range(NCORE)))
    return _assemble([res.results[c]["out"] for c in range(NCORE)])


# revision 38
# speedup vs baseline: 1.0011x; 1.0011x over previous
"""BloomBlock (B=1, S=2048, H=2048, NH=16) on 8 Trainium2 NeuronCores.

Megatron tensor-parallel: each core owns 2 attention heads and 1024 rows of
the 8192-wide MLP. LN1 (+transpose) is replicated; attention/MLP partial sums
are reduce-scattered over the sequence in bf16; LN2 runs on the local
sequence shard; normalized activations are all-gathered (transposed) for the
MLP. The final output stays sequence-sharded; the host reassembles.

All collectives are chunked by sequence groups of 512 rows (4 chunks) so
they overlap compute: attention+dense for group g is followed immediately by
ReduceScatter chunk g (hidden under group g+1's attention); LN2/AllGather
for the first two strips run mid-attention; fc1/fc2 run per group with RS2
chunk g issued behind fc2 group g. Row-chunked ReduceScatter gives core c
ownership of 4 strips of 64 rows (rows g*512 + c*64 + [0,64)); the host
maps res1/out accordingly.

All matmul operands are bf16 (fp32 PSUM accumulation); LayerNorm, softmax
statistics, alibi, residuals and final outputs are fp32.

Activations are kept transposed [feature, seq] so every matmul contracts
along the partition axis:
  hid --LN1--> xhat --PE transpose--> xT
  qT,kT = Wqk @ xT            v = xT.T @ WvT   (natural [s,d], lhsT for ctx)
  scores = qT.T @ kT  (+alibi fused w/ row-max, causal mask, ACT exp+rowsum)
  wT = probs.T @ diag(1/rowsum)   (PE transpose-normalize)
  ctxT = v.T @ wT  (+v bias)
  dense partial = ctxT.T @ WdT --> bf16 ReduceScatter --> + residual1
  LN2 --> yT strips --> bf16 AllGather --> hdnT = gelu(Wf1 @ yT + b)
  mlp partial = hdnT.T @ Wf2T --> bf16 ReduceScatter --> + residual2 --> out
"""
import sys

for _p in ("/opt/trn_rl_repo",):
    if _p not in sys.path:
        sys.path.insert(0, _p)

import numpy as np
import ml_dtypes

import concourse.bass as bass
from concourse import bacc
import concourse.mybir as mybir
import concourse.tile as tile
from concourse.bass_utils import run_bass_kernel_spmd
from concourse.masks import make_identity, make_causal_mask

AF = mybir.ActivationFunctionType
ALU = mybir.AluOpType
AX = mybir.AxisListType

B, S, H, NH, HD = 1, 2048, 2048, 16, 128
NCORE = 8
NHC = NH // NCORE          # heads per core = 2
SSH = S // NCORE           # sequence shard = 256
F1 = 4 * H                 # 8192
F1C = F1 // NCORE          # 1024
NSB = S // 128             # 16 s-blocks
NHCH = H // 128            # 16 h-chunks
NG = 5                     # sequence groups (collective chunks), uneven
GRP = [(0, 512), (512, 512), (1024, 512), (1536, 256), (1792, 256)]
STW = [w // NCORE for _, w in GRP]                        # strip widths
SB2G = []                   # s-block -> (group, local row offset)
for _sb in range(NSB):
    for _g, (_st, _w) in enumerate(GRP):
        if _st <= _sb * 128 < _st + _w:
            SB2G.append((_g, _sb * 128 - _st))
            break
EPS = 1e-5
MASK_NEG = -1e30

FP = mybir.dt.float32
BF = mybir.dt.bfloat16

SIM_MODE = False           # CoreSim lacks Gelu; use Identity there
PANEL = 256                # stage-A transpose/QKV panel width
NPANEL = S // PANEL


def _ln_stats(nc, stats, work, src, eps_t, scratch_tag="scsb",
              scratch=None):
    """Mean/rstd over the free axis of src [128, H]. Returns (rstd, -mu*rstd)."""
    ssum = stats.tile([128, 1], FP, tag="ssum")
    nc.vector.reduce_sum(ssum, src, axis=AX.X)
    mu = stats.tile([128, 1], FP, tag="mu")
    nc.vector.tensor_scalar_mul(mu, ssum, 1.0 / H)
    sq = scratch if scratch is not None else work.tile(
        [128, H], FP, tag=scratch_tag, bufs=2, name="sq")
    ssq = stats.tile([128, 1], FP, tag="ssq")
    nc.scalar.activation(sq, src, AF.Square, accum_out=ssq)
    msq = stats.tile([128, 1], FP, tag="msq")
    nc.vector.tensor_scalar_mul(msq, ssq, 1.0 / H)
    mu2 = stats.tile([128, 1], FP, tag="mu2")
    nc.vector.tensor_mul(mu2, mu, mu)
    var = stats.tile([128, 1], FP, tag="var")
    nc.vector.tensor_sub(var, msq, mu2)
    std = stats.tile([128, 1], FP, tag="std")
    nc.scalar.activation(std, var, AF.Sqrt, bias=eps_t)
    rstd = stats.tile([128, 1], FP, tag="rstd")
    nc.vector.reciprocal(rstd, std)
    nmurs = stats.tile([128, 1], FP, tag="nmurs")
    nc.vector.tensor_mul(nmurs, mu, rstd)
    nc.vector.tensor_scalar_mul(nmurs, nmurs, -1.0)
    return rstd, nmurs


def build_program():
    nc = bacc.Bacc("TRN2", target_bir_lowering=False, debug=False,
                   enable_asserts=True, num_devices=NCORE)

    # ---------------- I/O ----------------
    hid = nc.declare_dram_parameter("hid", [S, H], BF, isOutput=False)
    wqk = nc.declare_dram_parameter("wqk", [H, 4 * HD], BF, isOutput=False)
    bqk = nc.declare_dram_parameter("bqk", [4, HD], FP, isOutput=False)
    wv = nc.declare_dram_parameter("wv", [H, NHC * HD], BF, isOutput=False)
    bv = nc.declare_dram_parameter("bv", [NHC, HD], FP, isOutput=False)
    alib = nc.declare_dram_parameter("alib", [NHC, S], FP, isOutput=False)
    wd = nc.declare_dram_parameter("wd", [NHC * HD, H], BF, isOutput=False)
    res1 = nc.declare_dram_parameter("res1", [SSH, H], FP, isOutput=False)
    wf1 = nc.declare_dram_parameter("wf1", [H, F1C], BF, isOutput=False)
    bf1 = nc.declare_dram_parameter("bf1", [F1C // 128, 128], FP, isOutput=False)
    wf2 = nc.declare_dram_parameter("wf2", [F1C, H], BF, isOutput=False)
    fc2b = nc.declare_dram_parameter("fc2b", [128, H], BF, isOutput=False)
    out = nc.declare_dram_parameter("out", [SSH, H], FP, isOutput=True)

    rg = [list(range(NCORE))]

    with tile.TileContext(nc) as tc:
        with (
            tc.tile_pool(name="dram", bufs=1, space="DRAM") as dram,
            tc.tile_pool(name="consts", bufs=1) as consts,
            tc.tile_pool(name="stats", bufs=4) as stats,
            tc.tile_pool(name="pmm", bufs=1, space="PSUM") as pmm,
            tc.tile_pool(name="psc", bufs=2, space="PSUM") as psc,
            tc.tile_pool(name="psw", bufs=1, space="PSUM") as psw,
            tc.tile_pool(name="stageE", bufs=1) as stageE,
            tc.tile_pool(name="workE", bufs=2) as workE,
        ):
            # ---------------- constants ----------------
            ident = consts.tile([128, 128], BF, tag="ident")
            make_identity(nc, ident)
            cmask = consts.tile([128, 128], FP, tag="cmask")
            make_causal_mask(nc, cmask, mask_val=MASK_NEG)
            bqk_t = consts.tile([128, 4], FP, tag="bqk")
            nc.sync.dma_start(out=bqk_t, in_=bqk[:, :].rearrange("b p -> p b"))
            bv_t = consts.tile([128, NHC], FP, tag="bv")
            nc.sync.dma_start(out=bv_t, in_=bv[:, :].rearrange("b p -> p b"))
            bf1_t = consts.tile([128, F1C // 128], FP, tag="bf1")
            nc.sync.dma_start(out=bf1_t, in_=bf1[:, :].rearrange("b p -> p b"))
            eps_t = consts.tile([128, 1], FP, tag="eps")
            nc.vector.memset(eps_t, EPS)
            fc2b_t = consts.tile([128, H], BF, tag="fc2b")
            nc.sync.dma_start(out=fc2b_t, in_=fc2b[:, :])

            # collective bounce buffers, chunked by uneven sequence groups;
            # core c receives a STW[g]-row strip per chunk
            rs1_in = [dram.tile([GRP[g][1], H], BF, tag=f"rs1i{g}",
                                name=f"rs1i{g}") for g in range(NG)]
            rs1_out = [dram.tile([STW[g], H], BF, tag=f"rs1o{g}",
                                 name=f"rs1o{g}") for g in range(NG)]
            ag_in = [dram.tile([H, STW[g]], BF, tag=f"agi{g}", name=f"agi{g}")
                     for g in range(NG)]
            ag_out = [dram.tile([NCORE, H, STW[g]], BF, tag=f"ago{g}",
                                name=f"ago{g}", addr_space="Shared")
                      for g in range(NG)]
            rs2_in = [dram.tile([GRP[g][1], H], BF, tag=f"rs2i{g}",
                                name=f"rs2i{g}") for g in range(NG)]
            rs2_out = [dram.tile([STW[g], H], BF, tag=f"rs2o{g}",
                                 name=f"rs2o{g}") for g in range(NG)]

            # persistent stage-E state (coexists with attention pools)
            attn = stageE.tile([128, 2, H], FP, tag="attn")

            # tile 0 = group-0 strip (128 rows); tile 1 = strips 1,2,3
            TILE_GROUPS = [[0, 1], [2, 3, 4]]

            def emit_ln2_tile(t):
                """LN2 + transpose for the tile's strips; then AG chunks."""
                r1b = workE.tile([128, H], FP, tag="res1blk", name="r1b",
                                 bufs=1)
                nc.sync.dma_start(out=r1b,
                                  in_=res1[t * 128:(t + 1) * 128, :])
                rsb = workE.tile([128, H], BF, tag="rs1blk", name="rsb")
                po = 0
                for g in TILE_GROUPS[t]:
                    nc.sync.dma_start(out=rsb[po:po + STW[g], :],
                                      in_=rs1_out[g])
                    po += STW[g]
                nc.vector.tensor_add(attn[:, t, :], rsb, r1b)
                rstd, nmurs = _ln_stats(nc, stats, workE, attn[:, t, :],
                                        eps_t, scratch=r1b)
                yh = workE.tile([128, H], BF, tag="yhat", bufs=1)
                nc.scalar.activation(yh, attn[:, t, :], AF.Identity,
                                     bias=nmurs, scale=rstd)
                yT = workE.tile([128, NHCH, 128], BF, tag="yT", bufs=1)
                for hc in range(NHCH):
                    pt = pmm.tile([128, 128], BF, tag="mmT", bufs=1)
                    nc.tensor.transpose(pt, yh[:, hc * 128:(hc + 1) * 128],
                                        ident)
                    nc.scalar.copy(yT[:, hc, :], pt)
                po = 0
                for g in TILE_GROUPS[t]:
                    nc.sync.dma_start(
                        out=ag_in[g][:, :].rearrange("(c p) s -> p c s", p=128),
                        in_=yT[:, :, po:po + STW[g]])
                    po += STW[g]
                    nc.gpsimd.collective_compute(
                        "AllGather", ALU.bypass, replica_groups=rg,
                        ins=[ag_in[g].opt()], outs=[ag_out[g].opt()])

            # ======== attention-lifetime scope (stages A-D) ========
            with (
                tc.tile_pool(name="attnp", bufs=1) as attnp,
                tc.tile_pool(name="workA", bufs=2) as workA,
            ):
                # first panel's hidden blocks first so LN1 starts immediately
                hb_pre = []
                for sb in range(PANEL // 128):
                    hb = workA.tile([128, H], BF, tag="hidblk", name="hb")
                    nc.sync.dma_start(out=hb, in_=hid[sb * 128:(sb + 1) * 128, :])
                    hb_pre.append(hb)

                alibi_bc = attnp.tile([128, NHC, S], FP, tag="alibi")
                for h in range(NHC):
                    astage = workA.tile([128, S], FP, tag="scsb", name="astage")
                    nc.sync.dma_start(out=astage[0:1, :], in_=alib[h:h + 1, :])
                    nc.gpsimd.partition_broadcast(
                        alibi_bc[:, h, :], astage[0:1, :])
                # upper-bound row max: m[q] = alibi[q] + 8 (alibi rises in k,
                # so the true max of s+alibi is within ~8 of alibi[q])
                negm = attnp.tile([128, NHC * NSB], FP, tag="negm")
                acol = workA.tile([128, NHC * NSB], FP, tag="acol", bufs=1)
                nc.sync.dma_start(
                    out=acol,
                    in_=alib[:, :].rearrange("h (b p) -> p (h b)", p=128))
                nc.vector.tensor_scalar(negm, acol, -1.0, -8.0,
                                        ALU.mult, ALU.add)
                wd_t = attnp.tile([128, NHC, H], BF, tag="wd")
                for h in range(NHC):
                    nc.sync.dma_start(out=wd_t[:, h, :],
                                      in_=wd[h * 128:(h + 1) * 128, :])
                wqk_t = attnp.tile([128, NHCH, 4 * HD], BF, tag="wqk")
                wv_t = attnp.tile([128, NHCH, NHC * HD], BF, tag="wv")
                for hc in range(NHCH):
                    nc.sync.dma_start(out=wqk_t[:, hc, :],
                                      in_=wqk[hc * 128:(hc + 1) * 128, :])
                    nc.sync.dma_start(out=wv_t[:, hc, :],
                                      in_=wv[hc * 128:(hc + 1) * 128, :])

                qkT = attnp.tile([128, 2, NHC, S], BF, tag="qkT")
                v_t = attnp.tile([128, NSB, NHC * HD], BF, tag="v")
                ctxT = attnp.tile([128, NHC, S], BF, tag="ctxT")

                # ---- Stage A+B: LN1, transpose, QKV per panel ----
                for p in range(NPANEL):
                    xhat_blocks = []
                    for sb in range(PANEL // 128):
                        s0 = p * PANEL + sb * 128
                        if p == 0:
                            hb = hb_pre[sb]
                        else:
                            hb = workA.tile([128, H], BF, tag="hidblk",
                                            name="hb")
                            nc.sync.dma_start(out=hb, in_=hid[s0:s0 + 128, :])
                        rstd, nmurs = _ln_stats(nc, stats, workA, hb, eps_t)
                        xh = workA.tile([128, H], BF, tag="xhat", bufs=3)
                        nc.scalar.activation(xh, hb, AF.Identity,
                                             bias=nmurs, scale=rstd)
                        xhat_blocks.append(xh)

                    xT = workA.tile([128, NHCH, PANEL], BF, tag="xT", bufs=1)
                    for hc in range(NHCH):
                        pt = pmm.tile([128, PANEL], BF, tag="mmT", bufs=1)
                        for sb in range(PANEL // 128):
                            nc.tensor.transpose(
                                pt[:, sb * 128:(sb + 1) * 128],
                                xhat_blocks[sb][:, hc * 128:(hc + 1) * 128],
                                ident)
                        nc.vector.tensor_copy(xT[:, hc, :], pt)

                    for fb in range(4):  # q_h0, k_h0, q_h1, k_h1
                        pq = pmm.tile([128, PANEL], FP, tag="mm", bufs=3)
                        for hc in range(NHCH):
                            nc.tensor.matmul(
                                pq, wqk_t[:, hc, fb * 128:(fb + 1) * 128],
                                xT[:, hc, :],
                                start=(hc == 0), stop=(hc == NHCH - 1))
                        head, isk = fb // 2, fb % 2
                        nc.scalar.activation(
                            qkT[:, isk, head, p * PANEL:(p + 1) * PANEL], pq,
                            AF.Identity, bias=bqk_t[:, fb:fb + 1], scale=1.0)

                    for sb in range(PANEL // 128):
                        blk = p * (PANEL // 128) + sb
                        pv = pmm.tile([128, NHC * HD], FP, tag="mm", bufs=3)
                        for hc in range(NHCH):
                            nc.tensor.matmul(
                                pv, xT[:, hc, sb * 128:(sb + 1) * 128],
                                wv_t[:, hc, :],
                                start=(hc == 0), stop=(hc == NHCH - 1))
                        nc.vector.tensor_copy(v_t[:, blk, :], pv)

                # ---- Stage C+D: attention, dense, RS1 per group ----
                for sg in range(4):
                    for head in range(NHC):
                        probs_g = []
                        dn_g = []
                        for ib in range(4 * sg, 4 * sg + 4):
                            jw = (ib + 1) * 128
                            nchunk = (jw + 511) // 512
                            probs = workA.tile([128, S], BF, tag="probs", bufs=5)
                            ssc = workA.tile([128, S], FP, tag="scsb", bufs=2,
                                             name="ssc")
                            nmc = negm[:, head * NSB + ib:head * NSB + ib + 1]
                            dparts = []
                            for jc in range(nchunk):
                                j0 = jc * 512
                                w_ = min(512, jw - j0)
                                ps = psc.tile([128, 512], FP, tag="sc")
                                nc.tensor.matmul(
                                    ps[:, :w_],
                                    qkT[:, 0, head, ib * 128:(ib + 1) * 128],
                                    qkT[:, 1, head, j0:j0 + w_],
                                    start=True, stop=True)
                                if jc == nchunk - 1:
                                    off = ib * 128 - j0
                                    nc.vector.tensor_add(
                                        ps[:, off:off + 128],
                                        ps[:, off:off + 128], cmask)
                                nc.vector.tensor_add(
                                    ssc[:, j0:j0 + w_], ps[:, :w_],
                                    alibi_bc[:, head, j0:j0 + w_])
                                dpart = stats.tile([128, 1], FP,
                                                   tag=f"dp{jc}", name="dp")
                                nc.scalar.activation(
                                    probs[:, j0:j0 + w_], ssc[:, j0:j0 + w_],
                                    AF.Exp,
                                    bias=nmc, scale=1.0, accum_out=dpart)
                                dparts.append(dpart)
                            denom = dparts[0]
                            for dp in dparts[1:]:
                                dnew = stats.tile([128, 1], FP, tag="denom")
                                nc.vector.tensor_add(dnew, denom, dp)
                                denom = dnew
                            invd = stats.tile([128, 1], FP, tag="invd")
                            nc.vector.reciprocal(invd, denom)
                            dn = workA.tile([128, 128], BF, tag="dn", bufs=5)
                            nc.vector.tensor_scalar_mul(dn, ident, invd)
                            probs_g.append(probs)
                            dn_g.append(dn)

                        pctx = psw.tile([128, 512], FP, tag="ctxacc", bufs=1)
                        njc = 4 * sg + 4
                        for jc in range(njc):
                            ib0 = max(jc, 4 * sg)
                            nblk = 4 * sg + 4 - ib0
                            pw = psw.tile([128, 512], FP, tag="wt")
                            for k, ib in enumerate(range(ib0, 4 * sg + 4)):
                                nc.tensor.matmul(
                                    pw[:, k * 128:(k + 1) * 128],
                                    probs_g[ib - 4 * sg][:, jc * 128:(jc + 1) * 128],
                                    dn_g[ib - 4 * sg],
                                    start=True, stop=True)
                            wts = workA.tile([128, 512], BF, tag="wts", bufs=2)
                            nc.vector.tensor_copy(wts[:, :nblk * 128],
                                                  pw[:, :nblk * 128])
                            off = (ib0 - 4 * sg) * 128
                            nc.tensor.matmul(
                                pctx[:, off:off + nblk * 128],
                                v_t[:, jc, head * HD:(head + 1) * HD],
                                wts[:, :nblk * 128],
                                start=(jc == 0), stop=(jc == njc - 1),
                                skip_group_check=True)
                        nc.scalar.activation(
                            ctxT[:, head, sg * 512:(sg + 1) * 512], pctx,
                            AF.Identity, bias=bv_t[:, head:head + 1], scale=1.0)

                    # dense partials for this subgroup's 512 rows; RS1
                    # chunks fire as uneven groups complete
                    for sbl in range(4):
                        sb = 4 * sg + sbl
                        gi, loc = SB2G[sb]
                        for fc in range(H // 512):
                            pd = pmm.tile([128, 512], FP, tag="mm", bufs=3)
                            for h in range(NHC):
                                nc.tensor.matmul(
                                    pd, ctxT[:, h, sb * 128:(sb + 1) * 128],
                                    wd_t[:, h, fc * 512:(fc + 1) * 512],
                                    start=(h == 0), stop=(h == NHC - 1))
                            dsb = workA.tile([128, 512], BF, tag="densebf",
                                             bufs=4)
                            nc.scalar.copy(dsb, pd)
                            nc.sync.dma_start(
                                out=rs1_in[gi][loc:loc + 128,
                                               fc * 512:(fc + 1) * 512],
                                in_=dsb)
                        if (sb + 1) * 128 == GRP[gi][0] + GRP[gi][1] \
                                and gi < 3:
                            nc.gpsimd.collective_compute(
                                "ReduceScatter", ALU.add, replica_groups=rg,
                                ins=[rs1_in[gi].opt()],
                                outs=[rs1_out[gi].opt()])

                    if sg == 2:
                        # tile-0 strips landed after subgroup 1; LN2 tile 0
                        # + AG chunks run while subgroup 3 computes
                        emit_ln2_tile(0)

            # ======== Stage E tail + F: MLP per group + RS2 ========
            with (
                tc.tile_pool(name="postp", bufs=1) as postp,
                tc.tile_pool(name="workF", bufs=2) as workF,
            ):
                nf1 = F1C // 128  # 8
                wf1_t = postp.tile([128, NHCH, F1C], BF, tag="wf1")
                for hc in range(NHCH):
                    nc.sync.dma_start(out=wf1_t[:, hc, :],
                                      in_=wf1[hc * 128:(hc + 1) * 128, :])
                def emit_fc1_chunk(g, sc):
                    W, SW = GRP[g][1], STW[g]
                    wcs = min(512, W - sc * 512)
                    c0 = sc * 512 // SW
                    ncs = wcs // SW
                    yTs = workF.tile([128, NHCH, 512], BF, tag="yTs",
                                     name="yTs", bufs=2)
                    for hc in range(NHCH):
                        nc.sync.dma_start(
                            out=yTs[:, hc, 0:wcs].rearrange(
                                "p (k s) -> p k s", k=ncs),
                            in_=ag_out[g][c0:c0 + ncs,
                                          hc * 128:(hc + 1) * 128,
                                          :].rearrange("k p s -> p k s"))
                    hdn = workF.tile([128, nf1, 512], BF, tag="hdnT",
                                     name="hdn", bufs=2)
                    for f1c in range(nf1):
                        pf = pmm.tile([128, 512], FP, tag="mm", bufs=3)
                        for hc in range(NHCH):
                            nc.tensor.matmul(
                                pf[:, :wcs],
                                wf1_t[:, hc, f1c * 128:(f1c + 1) * 128],
                                yTs[:, hc, 0:wcs],
                                start=(hc == 0), stop=(hc == NHCH - 1))
                        nc.scalar.activation(
                            hdn[:, f1c, 0:wcs], pf[:, :wcs],
                            AF.Identity if SIM_MODE else AF.Gelu_apprx_tanh,
                            bias=bf1_t[:, f1c:f1c + 1], scale=1.0)
                    return hdn, wcs

                def emit_fc2_chunk(g, sc, hdn, wcs):
                    for kk in range(wcs // 128):
                        r0 = sc * 512 + kk * 128
                        for fc in range(H // 512):
                            pm = pmm.tile([128, 512], FP, tag="mm", bufs=3)
                            for f1c in range(nf1):
                                nc.tensor.matmul(
                                    pm,
                                    hdn[:, f1c, kk * 128:(kk + 1) * 128],
                                    wf2_t[:, f1c, fc * 512:(fc + 1) * 512],
                                    start=(f1c == 0), stop=(f1c == nf1 - 1))
                            msb = workF.tile([128, 512], BF, tag="mlpbf",
                                             bufs=3)
                            nc.vector.tensor_copy(msb, pm)
                            nc.sync.dma_start(
                                out=rs2_in[g][r0:r0 + 128,
                                              fc * 512:(fc + 1) * 512],
                                in_=msb)

                def emit_rs2(g):
                    nc.gpsimd.collective_compute(
                        "ReduceScatter", ALU.add, replica_groups=rg,
                        ins=[rs2_in[g].opt()], outs=[rs2_out[g].opt()])

                def emit_final(t):
                    rsb2 = workE.tile([128, H], BF, tag="rs1blk",
                                      name="rsb2")
                    po = 0
                    for g in TILE_GROUPS[t]:
                        nc.sync.dma_start(out=rsb2[po:po + STW[g], :],
                                          in_=rs2_out[g])
                        po += STW[g]
                    ob = workE.tile([128, H], FP, tag="outblk", bufs=1)
                    nc.vector.tensor_add(ob, rsb2, attn[:, t, :])
                    nc.vector.tensor_add(ob, ob, fc2b_t)
                    nc.sync.dma_start(out=out[t * 128:(t + 1) * 128, :],
                                      in_=ob)

                h, w = emit_fc1_chunk(0, 0)
                # tail RS1 chunks fire only after fc1(0,0)'s input DMAs are
                # enqueued: collective traffic shares the DMA queues
                for gi in (3, 4):
                    nc.gpsimd.collective_compute(
                        "ReduceScatter", ALU.add, replica_groups=rg,
                        ins=[rs1_in[gi].opt()], outs=[rs1_out[gi].opt()])
                wf2_t = postp.tile([128, nf1, H], BF, tag="wf2")
                for f1c in range(nf1):
                    nc.sync.dma_start(out=wf2_t[:, f1c, :],
                                      in_=wf2[f1c * 128:(f1c + 1) * 128, :])
                emit_ln2_tile(1)
                emit_fc2_chunk(0, 0, h, w)
                emit_rs2(0)
                for g in range(1, NG):
                    h, w = emit_fc1_chunk(g, 0)
                    emit_fc2_chunk(g, 0, h, w)
                    emit_rs2(g)
                    if g == 1:
                        emit_final(0)
                emit_final(1)
    nc.compile()
    return nc


def _host_prep(inputs):
    """Slice/fold weights per core. Returns list of per-core input maps."""
    bf16 = ml_dtypes.bfloat16
    hs = np.asarray(inputs["hidden_states"], np.float32).reshape(S, H)
    g1 = np.asarray(inputs["ln1_g"], np.float32)
    b1 = np.asarray(inputs["ln1_b"], np.float32)
    qkv_w = np.asarray(inputs["qkv_w"], np.float32)
    qkv_b = np.asarray(inputs["qkv_b"], np.float32)
    dense_w = np.asarray(inputs["dense_w"], np.float32)
    dense_b = np.asarray(inputs["dense_b"], np.float32)
    g2 = np.asarray(inputs["ln2_g"], np.float32)
    b2 = np.asarray(inputs["ln2_b"], np.float32)
    fc1_w = np.asarray(inputs["fc1_w"], np.float32)
    fc1_b = np.asarray(inputs["fc1_b"], np.float32)
    fc2_w = np.asarray(inputs["fc2_w"], np.float32)
    fc2_b = np.asarray(inputs["fc2_b"], np.float32)
    alibi = np.asarray(inputs["alibi"], np.float32).reshape(NH, S)

    inv = 1.0 / np.sqrt(np.float32(HD))
    in_maps = []
    for c in range(NCORE):
        heads = [NHC * c + i for i in range(NHC)]
        wqk_cols, bqk_rows, wv_cols, bv_rows = [], [], [], []
        for h in heads:
            qr = qkv_w[h * 3 * HD:h * 3 * HD + HD, :]
            kr = qkv_w[h * 3 * HD + HD:h * 3 * HD + 2 * HD, :]
            vr = qkv_w[h * 3 * HD + 2 * HD:h * 3 * HD + 3 * HD, :]
            qb = qkv_b[h * 3 * HD:h * 3 * HD + HD] + qr @ b1
            kb = qkv_b[h * 3 * HD + HD:h * 3 * HD + 2 * HD] + kr @ b1
            vb = qkv_b[h * 3 * HD + 2 * HD:h * 3 * HD + 3 * HD] + vr @ b1
            wqk_cols.append((qr * g1[None, :]).T * inv)
            wqk_cols.append((kr * g1[None, :]).T)
            bqk_rows.append(qb * inv)
            bqk_rows.append(kb)
            wv_cols.append((vr * g1[None, :]).T)
            bv_rows.append(vb)
        # core c's strips: rows GRP[g][0] + c*STW[g] + [0, STW[g])
        strips = np.concatenate(
            [hs[GRP[g][0] + c * STW[g]: GRP[g][0] + (c + 1) * STW[g], :]
             for g in range(NG)], axis=0)
        in_maps.append({
            "hid": hs.astype(bf16),
            "wqk": np.ascontiguousarray(
                np.concatenate(wqk_cols, axis=1)).astype(bf16),
            "bqk": np.ascontiguousarray(np.stack(bqk_rows, axis=0)),
            "wv": np.ascontiguousarray(
                np.concatenate(wv_cols, axis=1)).astype(bf16),
            "bv": np.ascontiguousarray(np.stack(bv_rows, axis=0)),
            "alib": np.ascontiguousarray(alibi[heads[0]:heads[-1] + 1, :]),
            "wd": np.ascontiguousarray(
                dense_w[:, heads[0] * HD:(heads[-1] + 1) * HD].T).astype(bf16),
            "res1": np.ascontiguousarray(strips + dense_b[None, :]),
            "wf1": np.ascontiguousarray(
                (fc1_w[c * F1C:(c + 1) * F1C, :] * g2[None, :]).T).astype(bf16),
            "bf1": np.ascontiguousarray(
                (fc1_b[c * F1C:(c + 1) * F1C]
                 + fc1_w[c * F1C:(c + 1) * F1C, :] @ b2
                 ).reshape(F1C // 128, 128)),
            "wf2": np.ascontiguousarray(
                fc2_w[:, c * F1C:(c + 1) * F1C].T).astype(bf16),
            "fc2b": np.ascontiguousarray(
                np.broadcast_to(fc2_b.astype(bf16), (128, H))),
        })
    return in_maps


def _assemble(shards):
    """Reassemble strip-owned shards (uneven groups)."""
    full = np.empty((S, H), np.float32)
    for c, sh in enumerate(shards):
        sh = np.asarray(sh, np.float32)
        po = 0
        for g in range(NG):
            full[GRP[g][0] + c * STW[g]: GRP[g][0] + (c + 1) * STW[g], :] = \
                sh[po:po + STW[g]]
            po += STW[g]
    return full.reshape(B, S, H)


_CACHED_NC = None


_WARMED = False


def kernel(**inputs) -> np.ndarray:
    global _CACHED_NC, _WARMED
    in_maps = _host_prep(inputs)
    if _CACHED_NC is None:
        _CACHED_NC = build_program()
    if not _WARMED:
        # first execution after NEFF load has been observed to race on
        # collective warmup; run once and discard
        run_bass_kernel_spmd(_CACHED_NC, in_maps, list(range(NCORE)))
        _WARMED = True
    res = run_bass_kernel_spmd(_CACHED_NC, in_maps, list(range(NCORE)))
    return _assemble([res.results[c]["out"] for c in range(NCORE)])
